# revision 4
# baseline (speedup 1.0000x reference)
"""GNN message-passing (gather + segment-sum) on 8 TRN2 NeuronCores.

Strategy (edge-parallel, destination-sharded, no collectives):
  - Host: assign each destination node to one of 8*50 (core, block) buckets
    with a degree-balanced greedy packer (<=128 nodes per bucket, lo/hi edge
    sums equalized). Each core's edges are grouped by destination block and
    split by source half (src < 25000 vs >= 25000, to fit int16 gather
    indices), laid out as two block-major tile streams (all lo, then all hi).
    x is stored as a bf16 table with 256 B row pitch ([50000, 128] bf16,
    cols 0:64 = bf16(x), rest pad) so the gather moves only 128 B/row while
    honoring the SWDGE 256 B stride-granule requirement.
  - Device (per core, SPMD, fully independent):
      for each window (a run of dst blocks):
        dma_gather  mlo[128e, WT, 64] = xlo[src]   (one big SWDGE call)
        dma_gather  mhi[128e, WT, 64] = xhi[src]
        for each dst block b in the window:
          per tile: DVE tensor_scalar sel[128e, 128] = (iota == rel[:,t])
                    PE matmul psum[128dst, 64] += sel.T @ msgs[:,t,:]
          Act copy  outT[:, b*64:(b+1)*64] = psum (bf16)
      one DMA of outT [128, 50*64] bf16 to DRAM.
  - Host: inverse-permute per-core outputs back to node order.

Collision-free by construction (no scatter-add): duplicate destinations are
combined by the one-hot matmul accumulation in PSUM. The one-hot build uses
tensor_scalar with a per-partition scalar (rel) so the DVE runs in 4x mode.
The gather is the bottleneck (~0.71 ns/row with 128 B descriptors across 16
DMA engines); desc-gen (Pool), sel build (DVE), and matmuls (PE) overlap
under it. bf16 messages keep rel-L2 error ~1e-3, well inside the 2e-2 gate.
"""

import numpy as np
import ml_dtypes

import concourse.tile as tile
from concourse import bacc, mybir
from concourse import bass_utils
from concourse.bass import MemorySpace
from concourse.ap_utils import ap_is_contiguous

N_NODES = 50000
D = 64
N_CORES = 8
NODES_PER_CORE = N_NODES // N_CORES  # 6250
BLOCK = 128
SRC_SPLIT = 25000
PAD_REL = 999.0  # dst_rel value for padding edges (matches no iota column)
W_TILES = 96  # max tiles per gather window (12288 idxs < 16368 FIFO cap)


def raw_dma_gather(engine, out_ap, in_ap, idxs_ap, num_idxs, elem_size,
                   elem_step, queue_num=0):
    """dma_gather allowing elem_size_bytes that is not a multiple of 256
    (the stride granule, elem_step bytes, must still be a 256 multiple).
    Same construction as bass.BassGpSimd.dma_gather, minus that assert."""
    assert idxs_ap.dtype == mybir.dt.int16
    assert in_ap.dtype == out_ap.dtype
    assert in_ap.space == MemorySpace.DRAM
    assert idxs_ap.space == MemorySpace.SBUF
    assert out_ap.space == MemorySpace.SBUF
    assert ap_is_contiguous(in_ap.ap[1:])
    assert ap_is_contiguous(out_ap.ap[1:])
    assert ap_is_contiguous(idxs_ap.ap[1:])
    assert in_ap.ap[-1][1] == elem_size
    assert out_ap.ap[-1][1] == elem_size
    assert in_ap.ap[0][0] == elem_step
    stride_bytes = elem_step * mybir.dt.size(in_ap.dtype)
    stride_bytes_256 = stride_bytes // 256
    assert stride_bytes % 256 == 0 and stride_bytes_256 < 256

    _in_ap = engine.lower_ap_dma(in_ap, for_custom_bir_dma=True)
    _idxs_ap = engine.lower_ap(idxs_ap)
    _out_ap = engine.lower_ap(out_ap)
    return engine.add_instruction(
        mybir.InstDMAGatherAnt(
            name=engine.bass.get_next_instruction_name(),
            ins=[
                *_in_ap,
                _idxs_ap,
                engine.lower_val_access(engine.to_reg(num_idxs)),
            ],
            outs=[_out_ap],
            transpose=False,
            num_idxs=num_idxs,
            elem_size=elem_size,
            stride_bytes_256=stride_bytes_256,
            gen_mode=0,
            single_packet=False,
            queue_num=queue_num,
            sbuf_tokens_per_rank=0,
            sbuf_free_dim_per_rank=0,
            sbuf_free_dim_pad_per_rank=0,
            sbuf_byte_offset=0,
        )
    )


def assign_nodes(deg_lo, deg_hi, n_cores, n_blocks, block):
    """Greedy 2D balanced assignment of nodes to (core, block) buckets.

    Nodes (sorted by degree, descending) go to the bucket with minimal
    max(lo_sum, hi_sum) that still has node capacity. This equalizes both
    the lo and hi edge counts across all buckets, minimizing tile padding.

    Returns (core_of, block_of, pos_of): per-node arrays.
    """
    n_nodes = len(deg_lo)
    nb = n_cores * n_blocks
    lo_s = np.zeros(nb)
    hi_s = np.zeros(nb)
    cnt = np.zeros(nb, np.int64)
    assign = np.empty(n_nodes, np.int64)
    order = np.argsort(-(deg_lo + deg_hi), kind="stable")
    full_penalty = np.zeros(nb)
    for n in order:
        score = np.maximum(lo_s + deg_lo[n], hi_s + deg_hi[n]) + full_penalty
        b = int(np.argmin(score))
        assign[n] = b
        lo_s[b] += deg_lo[n]
        hi_s[b] += deg_hi[n]
        cnt[b] += 1
        if cnt[b] >= block:
            full_penalty[b] = 1e18
    core_of = assign // n_blocks
    block_of = assign % n_blocks
    pos_of = np.empty(n_nodes, np.int64)
    fill = np.zeros(nb, np.int64)
    for n in order:
        b = assign[n]
        pos_of[n] = fill[b]
        fill[b] += 1
    return core_of, block_of, pos_of


def bin_edges(edge_index, n_cores, nodes_per_core, block, src_split,
              n_blocks=None):
    """Bin edges into (core, src-half, dst-block) buckets with balanced
    node-to-bucket assignment, pad to common per-(half, block) tile counts.

    Stream layout per core: [lo tiles of b0..b49][hi tiles of b0..b49].

    Returns:
      T_lo, T_hi: [n_blocks] int arrays, tiles (128 edges) per bucket
      src16: [n_cores, 128, tot_tiles*8] int16 gather indices
      dstrel: [n_cores, 128, tot_tiles] bfloat16 relative dst (position-major)
      node_loc: (core_of, block_of, pos_of) for output reassembly
    """
    dst = np.asarray(edge_index[0], dtype=np.int64)
    src = np.asarray(edge_index[1], dtype=np.int64)
    n_nodes = n_cores * nodes_per_core
    if n_blocks is None:
        n_blocks = -(-nodes_per_core // block) + 1  # one extra for packing slack

    half = (src >= src_split).astype(np.int64)
    deg_lo = np.bincount(dst[half == 0], minlength=n_nodes)
    deg_hi = np.bincount(dst[half == 1], minlength=n_nodes)
    core_of, block_of, pos_of = assign_nodes(
        deg_lo, deg_hi, n_cores, n_blocks, block
    )

    core = core_of[dst]
    blk = block_of[dst]
    rel = pos_of[dst].astype(np.float32)
    # sort key: (core, half, block) -> half-major streams per core
    bucket = (core * 2 + half) * n_blocks + blk

    order = np.argsort(bucket, kind="stable")
    src_s = src[order]
    rel_s = rel[order]

    counts = np.bincount(bucket, minlength=n_cores * 2 * n_blocks).reshape(
        n_cores, 2, n_blocks
    )
    # common (across cores) tile count per (half, block); at least 1
    T = np.maximum(1, -(-counts.max(axis=0) // 128))  # [2, n_blocks]
    T_lo, T_hi = T[0], T[1]
    tot_tiles = int(T_lo.sum() + T_hi.sum())
    tot_edges = tot_tiles * 128

    # tile-stream start (in padded edge positions) of each (half, block)
    pad_sizes = T.reshape(-1) * 128  # lo blocks then hi blocks
    pad_starts = np.zeros(2 * n_blocks, dtype=np.int64)
    pad_starts[1:] = np.cumsum(pad_sizes)[:-1]

    src_pad = np.zeros((n_cores, tot_edges), dtype=np.int16)
    rel_pad = np.full((n_cores, tot_edges), PAD_REL, dtype=np.float32)

    cum = counts.reshape(n_cores, -1).cumsum(axis=1)
    starts_real = np.zeros((n_cores, 2 * n_blocks), dtype=np.int64)
    starts_real[:, 1:] = cum[:, :-1]
    core_base = np.zeros(n_cores, dtype=np.int64)
    core_counts = counts.sum(axis=(1, 2))
    core_base[1:] = np.cumsum(core_counts)[:-1]

    for c in range(n_cores):
        cnts = counts[c].reshape(-1)
        for hb in range(2 * n_blocks):
            n = int(cnts[hb])
            if n == 0:
                continue
            s = int(core_base[c] + starts_real[c, hb])
            p = int(pad_starts[hb])
            sv = src_s[s : s + n]
            if hb >= n_blocks:
                sv = sv - src_split
            src_pad[c, p : p + n] = sv.astype(np.int16)
            rel_pad[c, p : p + n] = rel_s[s : s + n]

    # gather indices: wrapped in 16 partitions (idx i -> [i%16, i//16]),
    # replicated to all 8 gpsimd partition groups
    w = src_pad.reshape(n_cores, -1, 16).transpose(0, 2, 1)
    src16 = np.tile(w, (1, 8, 1)).copy()  # [n_cores, 128, tot_tiles*8]

    # dst_rel: edge position-major: position i -> [i%128, i//128]
    # float32: tensor_scalar's per-partition scalar read requires f32
    dstrel = (
        rel_pad.reshape(n_cores, -1, 128)
        .transpose(0, 2, 1)
        .copy()
    )  # [n_cores, 128, tot_tiles] f32

    return (
        T_lo.astype(int),
        T_hi.astype(int),
        src16,
        dstrel,
        (core_of, block_of, pos_of),
    )


def make_windows(T_lo, T_hi, w_tiles=W_TILES):
    """Greedy block ranges with both lo and hi tile sums <= w_tiles.
    Returns list of (b_start, b_end)."""
    n_blocks = len(T_lo)
    wins = []
    b0 = 0
    sl = sh = 0
    for b in range(n_blocks):
        tl, th = int(T_lo[b]), int(T_hi[b])
        if b > b0 and (sl + tl > w_tiles or sh + th > w_tiles):
            wins.append((b0, b))
            b0, sl, sh = b, 0, 0
        sl += tl
        sh += th
    wins.append((b0, n_blocks))
    return wins


def make_x_table(x):
    """x [N, 64] f32 -> [N, 128] bf16 with 256 B row pitch; cols 0:64 =
    bf16(x), cols 64:128 unused (zeros)."""
    t = np.zeros((x.shape[0], 128), dtype=ml_dtypes.bfloat16)
    t[:, :64] = x.astype(ml_dtypes.bfloat16)
    return t


def make_iota():
    return np.broadcast_to(
        np.arange(BLOCK, dtype=np.float32)[None, :], (128, BLOCK)
    ).astype(ml_dtypes.bfloat16).copy()


def build_program(T_lo, T_hi, n_rows, src_split, d=D, block=BLOCK, repeat=1,
                  msgs_bufs=3, sel_bufs=8, psum_bufs=8, w_tiles=W_TILES):
    """Build the SPMD Bass program for given per-(half, block) tile counts.

    repeat > 1 wraps the window loop in a hardware For_i loop running the
    identical computation `repeat` times (device-time measurement by
    wall-clock slope; results unchanged — iterations overwrite outputs).
    """
    n_blocks = len(T_lo)
    TL = int(T_lo.sum())
    tot_tiles = TL + int(T_hi.sum())
    off_lo = np.zeros(n_blocks, dtype=np.int64)
    off_lo[1:] = np.cumsum(T_lo)[:-1]
    off_hi = np.zeros(n_blocks, dtype=np.int64)
    off_hi[1:] = np.cumsum(T_hi)[:-1]
    off_hi += TL
    windows = make_windows(T_lo, T_hi, w_tiles)

    nc = bacc.Bacc(
        "TRN2",
        target_bir_lowering=False,
        debug=False,
        num_devices=N_CORES,
        num_swdge_queues=4,
    )
    xt = nc.dram_tensor("xt", [n_rows, 128], mybir.dt.bfloat16,
                        kind="ExternalInput")
    src16 = nc.dram_tensor(
        "src16", [128, tot_tiles * 8], mybir.dt.int16, kind="ExternalInput"
    )
    dstrel = nc.dram_tensor(
        "dstrel", [128, tot_tiles], mybir.dt.float32, kind="ExternalInput"
    )
    iota_in = nc.dram_tensor(
        "iota", [128, block], mybir.dt.bfloat16, kind="ExternalInput"
    )
    out = nc.dram_tensor("out", [128, n_blocks * d], mybir.dt.bfloat16,
                         kind="ExternalOutput")

    x_lo = xt.ap()[0:src_split, 0:d]
    x_hi = xt.ap()[src_split:n_rows, 0:d]

    with tile.TileContext(nc) as tc:
        with (
            tc.tile_pool(name="meta", bufs=1) as meta_pool,
            tc.tile_pool(name="mlo", bufs=msgs_bufs) as mlo_pool,
            tc.tile_pool(name="mhi", bufs=msgs_bufs) as mhi_pool,
            tc.tile_pool(name="sel", bufs=sel_bufs) as sel_pool,
            tc.tile_pool(name="obuf", bufs=1) as obuf_pool,
            tc.tile_pool(name="psum", bufs=psum_bufs, space="PSUM") as psum_pool,
        ):
            src_t = meta_pool.tile([128, tot_tiles * 8], mybir.dt.int16)
            nc.sync.dma_start(src_t[:], src16.ap())
            rel_t = meta_pool.tile([128, tot_tiles], mybir.dt.float32)
            nc.sync.dma_start(rel_t[:], dstrel.ap())
            iota_t = meta_pool.tile([128, block], mybir.dt.bfloat16)
            nc.sync.dma_start(iota_t[:], iota_in.ap())

            outbuf = obuf_pool.tile([128, n_blocks * d], mybir.dt.bfloat16)

            def body():
                q = 0
                for b0, b1 in windows:
                    wl0, wl1 = int(off_lo[b0]), int(off_lo[b1 - 1] + T_lo[b1 - 1])
                    wh0, wh1 = int(off_hi[b0]), int(off_hi[b1 - 1] + T_hi[b1 - 1])
                    nl, nh = wl1 - wl0, wh1 - wh0
                    mlo = mlo_pool.tile([128, w_tiles, d], mybir.dt.bfloat16,
                                        tag="mlo")
                    raw_dma_gather(
                        nc.gpsimd, mlo[:, 0:nl, :], x_lo,
                        src_t[:, wl0 * 8 : wl1 * 8], nl * 128, d, 2 * d,
                        queue_num=q % 4,
                    )
                    q += 1
                    mhi = mhi_pool.tile([128, w_tiles, d], mybir.dt.bfloat16,
                                        tag="mhi")
                    raw_dma_gather(
                        nc.gpsimd, mhi[:, 0:nh, :], x_hi,
                        src_t[:, wh0 * 8 : wh1 * 8], nh * 128, d, 2 * d,
                        queue_num=q % 4,
                    )
                    q += 1

                    for b in range(b0, b1):
                        tl, th = int(T_lo[b]), int(T_hi[b])
                        tb = tl + th
                        psum = psum_pool.tile([block, d], mybir.dt.float32,
                                              space="PSUM")
                        done = 0
                        for buf, boff, woff, tn in (
                            (mlo, int(off_lo[b]), wl0, tl),
                            (mhi, int(off_hi[b]), wh0, th),
                        ):
                            for t in range(tn):
                                g = boff + t  # global tile index
                                sel = sel_pool.tile([128, block],
                                                    mybir.dt.bfloat16,
                                                    tag="sel")
                                nc.vector.tensor_scalar(
                                    out=sel[:],
                                    in0=iota_t[:],
                                    scalar1=rel_t[:, g : g + 1],
                                    scalar2=None,
                                    op0=mybir.AluOpType.is_equal,
                                )
                                nc.tensor.matmul(
                                    out=psum[:],
                                    lhsT=sel[:],
                                    rhs=buf[:, g - woff, :],
                                    start=(done == 0),
                                    stop=(done == tb - 1),
                                )
                                done += 1
                        nc.scalar.mul(outbuf[:, b * d : (b + 1) * d], psum[:],
                                      1.0)
                nc.sync.dma_start(out.ap(), outbuf[:])

            if repeat > 1:
                with tc.For_i(0, repeat, 1):
                    body()
            else:
                body()

    nc.compile()
    return nc


def unshard_output(results, node_loc, block=BLOCK, n_nodes=N_NODES, d=D):
    core_of, block_of, pos_of = node_loc
    rows = block_of * block + pos_of
    out = np.empty((n_nodes, d), dtype=np.float32)
    for c in range(len(results)):
        r = np.asarray(results[c]["out"]).astype(np.float32)
        n_blocks = r.shape[1] // d
        r = r.reshape(128, n_blocks, d).transpose(1, 0, 2).reshape(-1, d)
        mask = core_of == c
        out[mask] = r[rows[mask]]
    return out


def kernel(edge_index, x):
    edge_index = np.asarray(edge_index)
    x = np.ascontiguousarray(np.asarray(x, dtype=np.float32))
    T_lo, T_hi, src16, dstrel, node_loc = bin_edges(
        edge_index, N_CORES, NODES_PER_CORE, BLOCK, SRC_SPLIT
    )
    nc = build_program(T_lo, T_hi, N_NODES, SRC_SPLIT)

    xt = make_x_table(x)
    iota = make_iota()
    in_maps = [
        {"xt": xt, "src16": src16[c], "dstrel": dstrel[c], "iota": iota}
        for c in range(N_CORES)
    ]
    res = bass_utils.run_bass_kernel_spmd(nc, in_maps, core_ids=list(range(N_CORES)))
    return unshard_output(res.results, node_loc)
